# revision 1
# baseline (speedup 1.0000x reference)
"""Trainium2 Bass kernel for a 2-layer GCN + global mean pool + MLP head.

Strategy (8 NeuronCores, SPMD):
  - Nodes (and their incident edges, grouped by destination) are sharded
    across the 8 cores; each core owns N/8 destination nodes.
  - GCN normalization is factored as D^-1/2 A D^-1/2 x = dinv * (A (dinv*x)),
    so gather tables are pre-scaled by dinv and aggregation is a plain
    segment-sum over dst-sorted edges (self-loops appended).
  - Layer 1 uses linearity: A_norm (x W1) == (A_norm x) W1, so the gather
    operates on the 14-dim (padded) input features, W1 applied afterwards.
  - Segment sums run on the tensor engine: gathered edge tiles [128e, F]
    are multiplied by per-tile one-hot matrices (edge -> local dst), which
    are generated on the vector engine via iota==dstloc compares, and
    accumulated in PSUM per 128-destination window.
  - Between layers, each core's shard of the (h1 @ W2) * dinv table is
    AllGather-ed so every core can gather arbitrary source rows.
  - Mean-pool partial sums are computed with the same one-hot-matmul trick
    (node -> graph id), AllReduce-summed, and the tiny MLP head plus
    log_softmax run redundantly on every core.
"""

import os
import numpy as np
import ml_dtypes

import concourse.bacc as bacc
import concourse.bass as bass
import concourse.mybir as mybir
import concourse.tile as tile
from concourse.bass_utils import run_bass_kernel_spmd

F32 = mybir.dt.float32
F8 = mybir.dt.float8e4
BF16 = mybir.dt.bfloat16
I16 = mybir.dt.int16
AF = mybir.ActivationFunctionType
OP = mybir.AluOpType
NPBF16 = ml_dtypes.bfloat16
NPF8 = ml_dtypes.float8_e4m3

SELU_LAM = 1.0507009873554805
SELU_ALPHA = 1.6732632423543772
SELU_LA = SELU_LAM * SELU_ALPHA

P = 128
NCORES = 8
SPLIT = 32768  # int16 index limit for dma_gather


class Cfg:
    def __init__(self, n_nodes, n_graphs, d_in, d_hid, d_fc, n_cls, nlo_cw, nhi_cw, grp):
        assert n_nodes % NCORES == 0
        self.N = n_nodes
        self.G = n_graphs
        self.DIN = d_in
        self.DH = d_hid          # 256
        self.DFC = d_fc          # 128
        self.NCLS = n_cls        # 2
        self.NSH = n_nodes // NCORES
        self.W = -(-self.NSH // P)          # windows per core
        self.GRP = grp                      # windows per gather group
        self.DPAD = 128                     # padded layer-1 feature cols (bf16)
        # groups: list of (first_window, n_windows)
        self.groups = []
        w = 0
        while w < self.W:
            wg = min(grp, self.W - w)
            self.groups.append((w, wg))
            w += wg
        # per-group tile budgets: max over cores and the group's windows
        # (static per call-site, identical across cores -> SPMD-safe)
        self.T_LO, self.T_HI, self.T = [], [], []
        for (w0, wg) in self.groups:
            tl = max(-(-int(nlo_cw[c, w]) // P)
                     for c in range(NCORES) for w in range(w0, w0 + wg))
            th = max(-(-int(nhi_cw[c, w]) // P)
                     for c in range(NCORES) for w in range(w0, w0 + wg))
            self.T_LO.append(tl)
            self.T_HI.append(th)
            self.T.append(tl + th)
        # slot/col layout per group (shared by both layers)
        self.grp_slot_base = []
        self.grp_lo_col = []   # idx slab col offsets (int16 cols, /16)
        self.grp_hi_col = []
        slot = 0
        col = 0
        for g, (w0, wg) in enumerate(self.groups):
            self.grp_slot_base.append(slot)
            self.grp_lo_col.append(col)
            col += wg * self.T_LO[g] * 8    # (wg*T_LO*128)/16 cols
            self.grp_hi_col.append(col)
            col += wg * self.T_HI[g] * 8
            slot += wg * self.T[g]
        self.SLOT_TOT = slot
        self.IDX_COLS = col
        self.T_MAX = max(self.T)


def _win_slots(cfg, g, k):
    """Slots (within the whole slot space) of window k of group g: lo tiles then hi tiles."""
    base = cfg.grp_slot_base[g]
    wg = cfg.groups[g][1]
    lo = [base + k * cfg.T_LO[g] + t for t in range(cfg.T_LO[g])]
    hi = [base + wg * cfg.T_LO[g] + k * cfg.T_HI[g] + t for t in range(cfg.T_HI[g])]
    return lo + hi


def host_prep(inputs, cfg):
    """Build per-core input maps + the static gather/one-hot metadata."""
    N, G = cfg.N, cfg.G
    x = np.asarray(inputs["x"], np.float32)
    ei = np.asarray(inputs["edge_index"], np.int64)
    batch = np.asarray(inputs["batch"], np.int64)

    src = np.concatenate([ei[0], np.arange(N, dtype=np.int64)])
    dst = np.concatenate([ei[1], np.arange(N, dtype=np.int64)])
    deg = np.bincount(dst, minlength=N).astype(np.float64)
    dinv = (1.0 / np.sqrt(np.maximum(deg, 1.0))).astype(np.float32)

    order = np.argsort(dst, kind="stable")
    s = src[order].astype(np.int64)
    d = dst[order].astype(np.int64)

    # layer-1 gather table: (x * dinv) padded to DPAD cols, bf16
    xt = np.zeros((N, cfg.DPAD), NPBF16)
    xt[:, : cfg.DIN] = (x * dinv[:, None]).astype(NPBF16)

    # per (core, window) edge slices
    W = cfg.W
    bounds = []
    for c in range(NCORES):
        for w in range(W):
            bounds.append(c * cfg.NSH + w * P)
    bounds.append(N)
    cut = np.searchsorted(d, np.asarray(bounds))

    per_core = []
    cnt = np.bincount(batch, minlength=G).astype(np.float32)
    cntinv = (1.0 / np.maximum(cnt, 1.0)).astype(np.float32)

    for c in range(NCORES):
        idx_slab = np.zeros((16, cfg.IDX_COLS), np.int16)
        dstloc = np.full((P, cfg.SLOT_TOT), -1.0, np.float32)
        for g, (w0, wg) in enumerate(cfg.groups):
            lo_stream = []
            hi_stream = []
            for k in range(wg):
                w = w0 + k
                a, b = cut[c * W + w], cut[c * W + w + 1]
                sw = s[a:b]
                dw = (d[a:b] - (c * cfg.NSH + w * P)).astype(np.int64)
                m = sw < SPLIT
                slo, dlo = sw[m], dw[m]
                shi, dhi = sw[~m] - SPLIT, dw[~m]
                tl, th = cfg.T_LO[g], cfg.T_HI[g]
                assert len(slo) <= tl * P and len(shi) <= th * P
                il = np.zeros(tl * P, np.int16)
                il[: len(slo)] = slo.astype(np.int16)
                ih = np.zeros(th * P, np.int16)
                ih[: len(shi)] = shi.astype(np.int16)
                lo_stream.append(il)
                hi_stream.append(ih)
                slots = _win_slots(cfg, g, k)
                dl = np.full((tl + th) * P, -1.0, np.float32)
                dl[: len(dlo)] = dlo
                dl[tl * P : tl * P + len(dhi)] = dhi
                for t in range(tl + th):
                    dstloc[:, slots[t]] = dl[t * P : (t + 1) * P]
            for stream, col0 in ((lo_stream, cfg.grp_lo_col[g]), (hi_stream, cfg.grp_hi_col[g])):
                flat = np.concatenate(stream) if stream else np.zeros(0, np.int16)
                if len(flat):
                    idx_slab[:, col0 : col0 + len(flat) // 16] = flat.reshape(-1, 16).T

        # per-window dinv / batch-id per partition
        dinv_w = np.zeros((P, W), np.float32)
        batchloc = np.full((P, W), -1.0, np.float32)
        base = c * cfg.NSH
        for w in range(W):
            rows = min(P, cfg.NSH - w * P)
            dinv_w[:rows, w] = dinv[base + w * P : base + w * P + rows]
            batchloc[:rows, w] = batch[base + w * P : base + w * P + rows].astype(np.float32)
        ohg = (batchloc[:, :, None] == np.arange(G, dtype=np.float32)[None, None, :])
        ohg = ohg.astype(NPBF16).reshape(P, W * G)

        oht = (dstloc[:, :, None] == np.arange(P, dtype=np.float32)[None, None, :])
        oht = oht.astype(NPF8).reshape(P, cfg.SLOT_TOT * P)
        im = {
            "xt": xt,
            "idxs": np.tile(idx_slab, (8, 1)),
            "oht": oht,
            "ohgt": ohg,
            "dinv_w": dinv_w,
        }
        per_core.append(im)

    # shared constants
    iota256 = np.tile(np.arange(G, dtype=np.float32)[None, :], (P, 1))
    W1p = np.zeros((16, cfg.DH), np.float32)
    W1p[: cfg.DIN] = np.asarray(inputs["W1"], np.float32)
    W2 = np.asarray(inputs["W2"], np.float32)
    W2_sb = np.concatenate([W2[:P, :], W2[P:, :]], axis=1)  # [128, 2*DH]
    b1 = np.asarray(inputs["b1"], np.float32).reshape(2, P).T.copy()  # [128,2] halves
    b2b = np.tile(np.asarray(inputs["b2"], np.float32)[None, :], (P, 1))
    fc1 = np.asarray(inputs["fc1_w"], np.float32)  # [DH, DFC]
    fc1_sb = np.concatenate([fc1[:P, :], fc1[P:, :]], axis=1)  # [128, 2*DFC]
    fc1b = np.asarray(inputs["fc1_b"], np.float32).reshape(P, 1)
    fc2 = np.asarray(inputs["fc2_w"], np.float32)  # [DFC, NCLS]
    fc2b = np.zeros((P, 1), np.float32)
    fc2b[: cfg.NCLS, 0] = np.asarray(inputs["fc2_b"], np.float32)
    ident = np.eye(P, dtype=np.float32)
    cntinv2 = np.tile(cntinv[None, :], (P, 2))  # [128, 2*G]

    shared = {
        "iota256": iota256,
        "W1p": W1p,
        "W2_sb": W2_sb,
        "b1h": b1,
        "b2b": b2b,
        "fc1_sb": fc1_sb,
        "fc1b": fc1b,
        "fc2_sb": fc2,
        "fc2b": fc2b,
        "ident": ident,
        "cntinv2": cntinv2,
        "has_b2": bool(np.any(b2b)),
    }
    for im in per_core:
        for k, v in shared.items():
            if k != "has_b2":
                im[k] = v
    return per_core, shared


def build_nc(cfg, has_b2):
    TMAX = cfg.T_MAX
    nc = bacc.Bacc("TRN2", target_bir_lowering=False, debug=False, num_devices=NCORES,
                   num_swdge_queues=2)
    N, G, W = cfg.N, cfg.G, cfg.W
    DH, DPAD = cfg.DH, cfg.DPAD

    xt = nc.dram_tensor("xt", [N, DPAD], BF16, kind="ExternalInput")
    idxs = nc.dram_tensor("idxs", [P, cfg.IDX_COLS], I16, kind="ExternalInput")
    oht_d = nc.dram_tensor("oht", [P, cfg.SLOT_TOT * P], F8, kind="ExternalInput")
    dinv_d = nc.dram_tensor("dinv_w", [P, W], F32, kind="ExternalInput")
    ohgt_d = nc.dram_tensor("ohgt", [P, W * G], BF16, kind="ExternalInput")
    W1p_d = nc.dram_tensor("W1p", [16, DH], F32, kind="ExternalInput")
    W2_d = nc.dram_tensor("W2_sb", [P, 2 * DH], F32, kind="ExternalInput")
    b1_d = nc.dram_tensor("b1h", [P, 2], F32, kind="ExternalInput")
    b2b_d = nc.dram_tensor("b2b", [P, DH], F32, kind="ExternalInput")
    fc1_d = nc.dram_tensor("fc1_sb", [P, 2 * cfg.DFC], F32, kind="ExternalInput")
    fc1b_d = nc.dram_tensor("fc1b", [P, 1], F32, kind="ExternalInput")
    fc2_d = nc.dram_tensor("fc2_sb", [cfg.DFC, cfg.NCLS], F32, kind="ExternalInput")
    fc2b_d = nc.dram_tensor("fc2b", [P, 1], F32, kind="ExternalInput")
    ident_d = nc.dram_tensor("ident", [P, P], F32, kind="ExternalInput")
    cntinv2_d = nc.dram_tensor("cntinv2", [P, 2 * G], F32, kind="ExternalInput")

    out_d = nc.dram_tensor("out", [G, cfg.NCLS], F32, kind="ExternalOutput")

    shard2 = nc.dram_tensor("shard2", [cfg.NSH, DH], BF16)
    h2t = nc.dram_tensor("h2t", [N, DH], BF16, addr_space="Shared")
    pool_part = nc.dram_tensor("pool_part", [2 * P, G], F32)
    pool_sum = nc.dram_tensor("pool_sum", [2 * P, G], F32, addr_space="Shared")

    negla_tile = [None]

    def selu_chain(vec, scal, dst_sb, zb_ap, e_ap, tmp_pool, shape, dt=F32):
        """dst = selu(zb) given e = exp(zb) already computed. The >0 mask is
        built on ACT (Sign then Relu) to keep DVE off the contended SBUF
        port while SWDGE descriptor generation runs; DVE only does the
        final predicated overwrite."""
        sgn = tmp_pool.tile(shape, F32, tag="selu_sgn")
        scal.activation(sgn[:], zb_ap, AF.Sign)
        mask = tmp_pool.tile(shape, mybir.dt.uint8, tag="selu_mask")
        scal.activation(mask[:], sgn[:], AF.Relu)
        # dst = λα e − λα
        scal.activation(dst_sb, e_ap, AF.Identity,
                        bias=negla_tile[0][:, 0:1], scale=SELU_LA)
        pos = tmp_pool.tile(shape, dt, tag="selu_pos")
        scal.activation(pos[:], zb_ap, AF.Identity, bias=0.0, scale=SELU_LAM)
        vec.copy_predicated(dst_sb, mask[:], pos[:])

    with tile.TileContext(nc) as tc:
        with (
            tc.tile_pool(name="consts", bufs=1) as cpool,
            tc.tile_pool(name="idxpool", bufs=1) as ipool,
            tc.tile_pool(name="gx1", bufs=2) as gx1pool,
            tc.tile_pool(name="gx2", bufs=2) as gx2pool,
            tc.tile_pool(name="oh", bufs=2) as ohpool,
            tc.tile_pool(name="work", bufs=3) as wpool,
            tc.tile_pool(name="head", bufs=1) as hpool,
            tc.tile_pool(name="post", bufs=2) as ppool,
            tc.tile_pool(name="ps_agg", bufs=2, space="PSUM") as ps_agg,
            tc.tile_pool(name="ps_tr", bufs=1, space="PSUM") as ps_tr,
            tc.tile_pool(name="ps_h1", bufs=1, space="PSUM") as ps_h1,
            tc.tile_pool(name="ps_h2", bufs=2, space="PSUM") as ps_h2,
            tc.tile_pool(name="ps_pool", bufs=1, space="PSUM") as ps_pool,
        ):
            # ---- load constants ----
            def load(pool, dram, shape, dt):
                t = pool.tile(shape, dt, tag=dram.name + "_sb")
                nc.sync.dma_start(out=t[:], in_=dram[tuple(slice(0, s) for s in shape)])
                return t

            negla = cpool.tile([P, 1], F32, tag="negla")
            nc.vector.memset(negla[:], -SELU_LA)
            negla_tile[0] = negla

            idx_sb = load(ipool, idxs, [P, cfg.IDX_COLS], I16)
            dinv_sb = load(cpool, dinv_d, [P, W], F32)
            W1p_sb = load(cpool, W1p_d, [16, DH], F32)
            W2_sb = load(cpool, W2_d, [P, 2 * DH], F32)
            b1_sb = load(cpool, b1_d, [P, 2], F32)
            b2b_sb = load(cpool, b2b_d, [P, DH], F32)
            fc1_sb = load(cpool, fc1_d, [P, 2 * cfg.DFC], F32)
            fc1b_sb = load(cpool, fc1b_d, [P, 1], F32)
            fc2_sb = load(cpool, fc2_d, [cfg.DFC, cfg.NCLS], F32)
            fc2b_sb = load(cpool, fc2b_d, [P, 1], F32)
            ident_sb = load(cpool, ident_d, [P, P], F32)
            cntinv2_sb = load(cpool, cntinv2_d, [P, 2 * G], F32)

            def load_onehots(g):
                w0, wg = cfg.groups[g]
                base = cfg.grp_slot_base[g]
                ohsl = ohpool.tile([P, wg * cfg.T[g] * P], F8, tag="ohslab")
                nc.sync.dma_start(
                    out=ohsl[:],
                    in_=oht_d[:, base * P : (base + wg * cfg.T[g]) * P],
                )
                return ohsl

            def gather_group(pool, g, table, elem, tag):
                w0, wg = cfg.groups[g]
                gt = pool.tile([P, wg * cfg.T[g], elem], BF16, tag=tag)
                nlo = wg * cfg.T_LO[g] * P
                if nlo:
                    nc.gpsimd.dma_gather(
                        gt[:, 0 : wg * cfg.T_LO[g], :],
                        table[0 : min(SPLIT, cfg.N), :],
                        idx_sb[:, cfg.grp_lo_col[g] : cfg.grp_lo_col[g] + nlo // 16],
                        nlo, nlo, elem, single_packet=False, queue_num=0,
                    )
                nhi = wg * cfg.T_HI[g] * P
                if nhi:
                    nc.gpsimd.dma_gather(
                        gt[:, wg * cfg.T_LO[g] : wg * cfg.T[g], :],
                        table[SPLIT : cfg.N, :],
                        idx_sb[:, cfg.grp_hi_col[g] : cfg.grp_hi_col[g] + nhi // 16],
                        nhi, nhi, elem, single_packet=False, queue_num=1,
                    )
                return gt


            # ================= Phase A: layer 1 -> shard2 =================
            for g, (w0, wg) in enumerate(cfg.groups):
                gt = gather_group(gx1pool, g, xt, DPAD, "gx1_t")
                ohsl = load_onehots(g)
                base = cfg.grp_slot_base[g]
                for k in range(wg):
                    w = w0 + k
                    slots = _win_slots(cfg, g, k)
                    psum1 = ps_agg.tile([P, 16], F32, tag="psum1")
                    Tg = cfg.T[g]
                    for t in range(Tg):
                        s = slots[t] - base
                        nc.tensor.matmul(
                            psum1[:], ohsl[:, s * P : (s + 1) * P], gt[:, s, 0:16],
                            start=(t == 0), stop=(t == Tg - 1),
                        )
                    aggx = wpool.tile([P, 16], F32, tag="aggx")
                    nc.scalar.activation(
                        aggx[:], psum1[:], AF.Copy, scale=dinv_sb[:, w : w + 1]
                    )
                    psT = ps_tr.tile([16, P], F32, tag="sm")
                    nc.tensor.transpose(psT[:], aggx[:], ident_sb[:])
                    aggxT = wpool.tile([16, P], F32, tag="aggxT")
                    nc.scalar.copy(aggxT[:], psT[:])
                    psum_h2t = ps_h2.tile([P, DH], F32, tag="main")
                    for j in range(2):
                        ph1 = ps_h1.tile([P, P], F32, tag="ph1")
                        nc.tensor.matmul(
                            ph1[:], W1p_sb[:, j * P : (j + 1) * P], aggxT[:],
                            start=True, stop=True,
                        )
                        e = ppool.tile([P, P], F32, tag="l1_e")
                        zb = ppool.tile([P, P], F32, tag="l1_zb")
                        nc.scalar.activation(e[:], ph1[:], AF.Exp, bias=b1_sb[:, j : j + 1])
                        nc.scalar.activation(zb[:], ph1[:], AF.Identity, bias=b1_sb[:, j : j + 1])
                        h1T = ppool.tile([P, P], F32, tag="l1_h1T")
                        selu_chain(nc.vector, nc.scalar, h1T[:], zb[:], e[:], ppool, [P, P])
                        nc.tensor.matmul(
                            psum_h2t[:], h1T[:], W2_sb[:, j * DH : (j + 1) * DH],
                            start=(j == 0), stop=(j == 1),
                        )
                    h2tw = ppool.tile([P, DH], BF16, tag="h2tw")
                    nc.scalar.activation(
                        h2tw[:], psum_h2t[:], AF.Copy, scale=dinv_sb[:, w : w + 1]
                    )
                    rows = min(P, cfg.NSH - w * P)
                    nc.sync.dma_start(
                        out=shard2[w * P : w * P + rows, :], in_=h2tw[:rows, :]
                    )

            # ================= AllGather h2 table =================
            nc.gpsimd.collective_compute(
                "AllGather", OP.bypass,
                replica_groups=[list(range(NCORES))],
                ins=[shard2[:, :]], outs=[h2t[:, :]],
            )

            # ================= Phase B: layer 2 + pooling =================
            pp0 = ps_pool.tile([P, G], F32, tag="pp0")
            pp1 = ps_pool.tile([P, G], F32, tag="pp1")
            pps = [pp0, pp1]
            for g, (w0, wg) in enumerate(cfg.groups):
                gt2 = gather_group(gx2pool, g, h2t, DH, "gx2_t")
                ohsl = load_onehots(g)
                ohg_sl = ohpool.tile([P, wg * G], BF16, tag="ohg_slab")
                nc.sync.dma_start(out=ohg_sl[:], in_=ohgt_d[:, w0 * G : (w0 + wg) * G])
                base = cfg.grp_slot_base[g]
                for k in range(wg):
                    w = w0 + k
                    slots = _win_slots(cfg, g, k)
                    psum2 = ps_h2.tile([P, DH], F32, tag="main")
                    Tg = cfg.T[g]
                    for t in range(Tg):
                        s = slots[t] - base
                        nc.tensor.matmul(
                            psum2[:], ohsl[:, s * P : (s + 1) * P], gt2[:, s, :],
                            start=(t == 0), stop=(t == Tg - 1),
                        )
                    zd = ppool.tile([P, DH], F32, tag="l2_zd")
                    nc.scalar.activation(
                        zd[:], psum2[:], AF.Copy, scale=dinv_sb[:, w : w + 1]
                    )
                    if has_b2:
                        zb2 = ppool.tile([P, DH], F32, tag="l2_zb")
                        nc.vector.tensor_tensor(zb2[:], zd[:], b2b_sb[:], OP.add)
                        zd = zb2
                    e2 = ppool.tile([P, DH], F32, tag="l2_e")
                    nc.scalar.activation(e2[:], zd[:], AF.Exp)
                    h2w = ppool.tile([P, DH], BF16, tag="l2_h2w")
                    selu_chain(nc.vector, nc.scalar, h2w[:], zd[:], e2[:], ppool,
                               [P, DH], dt=BF16)
                    for j in range(2):
                        nc.tensor.matmul(
                            pps[j][:], h2w[:, j * P : (j + 1) * P],
                            ohg_sl[:, k * G : (k + 1) * G],
                            start=(w == 0), stop=(w == W - 1),
                        )

            # ================= pooled head =================
            pT = hpool.tile([P, 2 * G], F32, tag="pT")
            nc.scalar.copy(pT[:, 0:G], pp0[:])
            nc.scalar.copy(pT[:, G : 2 * G], pp1[:])
            nc.sync.dma_start(out=pool_part[0:P, :], in_=pT[:, 0:G])
            nc.sync.dma_start(out=pool_part[P : 2 * P, :], in_=pT[:, G : 2 * G])
            nc.gpsimd.collective_compute(
                "AllReduce", OP.add,
                replica_groups=[list(range(NCORES))],
                ins=[pool_part[:, :]], outs=[pool_sum[:, :]],
            )
            ps = hpool.tile([P, 2 * G], F32, tag="ps_in")
            nc.sync.dma_start(out=ps[:, 0:G], in_=pool_sum[0:P, :])
            nc.sync.dma_start(out=ps[:, G : 2 * G], in_=pool_sum[P : 2 * P, :])
            pm = hpool.tile([P, 2 * G], F32, tag="pm")
            nc.vector.tensor_tensor(pm[:], ps[:], cntinv2_sb[:], OP.mult)
            eg = hpool.tile([P, 2 * G], F32, tag="eg")
            nc.scalar.activation(eg[:], pm[:], AF.Exp)
            gsel = hpool.tile([P, 2 * G], F32, tag="gsel")
            selu_chain(nc.vector, nc.scalar, gsel[:], pm[:], eg[:], hpool, [P, 2 * G])

            psum_fc1 = ps_h2.tile([P, G], F32, tag="main")
            for j in range(2):
                nc.tensor.matmul(
                    psum_fc1[:], fc1_sb[:, j * cfg.DFC : (j + 1) * cfg.DFC],
                    gsel[:, j * G : (j + 1) * G],
                    start=(j == 0), stop=(j == 1),
                )
            e3 = hpool.tile([P, G], F32, tag="e3")
            zb3 = hpool.tile([P, G], F32, tag="zb3")
            nc.scalar.activation(e3[:], psum_fc1[:], AF.Exp, bias=fc1b_sb[:, 0:1])
            nc.scalar.activation(zb3[:], psum_fc1[:], AF.Identity, bias=fc1b_sb[:, 0:1])
            hsel = hpool.tile([P, G], F32, tag="hsel")
            selu_chain(nc.vector, nc.scalar, hsel[:], zb3[:], e3[:], hpool, [P, G])

            psum_fc2 = ps_tr.tile([cfg.NCLS, G], F32, tag="sm")
            nc.tensor.matmul(psum_fc2[:], fc2_sb[:], hsel[:], start=True, stop=True)
            lg2 = wpool.tile([cfg.NCLS, G], F32, tag="lg2")
            nc.scalar.activation(
                lg2[:], psum_fc2[:], AF.Identity, bias=fc2b_sb[0 : cfg.NCLS, 0:1]
            )
            for j in range(-(-G // P)):
                gw = min(P, G - j * P)
                psT2 = ps_tr.tile([P, cfg.NCLS], F32, tag="sm")
                nc.tensor.transpose(
                    psT2[:gw, :], lg2[:, j * P : j * P + gw],
                    ident_sb[0 : cfg.NCLS, 0 : cfg.NCLS],
                )
                lgj = hpool.tile([P, cfg.NCLS], F32, tag="lgj")
                nc.scalar.copy(lgj[:gw, :], psT2[:gw, :])
                nm = hpool.tile([P, 1], F32, tag="nm")
                nc.vector.tensor_reduce(
                    nm[:gw, :], lgj[:gw, :], mybir.AxisListType.X, OP.max, negate=True
                )
                e4 = hpool.tile([P, cfg.NCLS], F32, tag="e4")
                nc.scalar.activation(e4[:gw, :], lgj[:gw, :], AF.Exp, bias=nm[:gw, 0:1])
                s4 = hpool.tile([P, 1], F32, tag="s4")
                nc.vector.tensor_reduce(s4[:gw, :], e4[:gw, :], mybir.AxisListType.X, OP.add)
                ls = hpool.tile([P, 1], F32, tag="ls")
                nc.scalar.activation(ls[:gw, :], s4[:gw, :], AF.Ln)
                q = hpool.tile([P, 1], F32, tag="q")
                nc.vector.tensor_tensor(q[:gw, :], nm[:gw, :], ls[:gw, :], OP.subtract)
                outj = hpool.tile([P, cfg.NCLS], F32, tag="outj")
                nc.vector.tensor_scalar(outj[:gw, :], lgj[:gw, :], q[:gw, 0:1], None, OP.add)
                nc.sync.dma_start(out=out_d[j * P : j * P + gw, :], in_=outj[:gw, :])

    nc.compile()
    return nc


def compute_tile_budget(inputs, n_nodes, nsh):
    """Per-(core, window) lo/hi edge counts."""
    ei = np.asarray(inputs["edge_index"], np.int64)
    N = n_nodes
    src = np.concatenate([ei[0], np.arange(N, dtype=np.int64)])
    dst = np.concatenate([ei[1], np.arange(N, dtype=np.int64)])
    order = np.argsort(dst, kind="stable")
    s, d = src[order], dst[order]
    W = -(-nsh // P)
    bounds = [c * nsh + w * P for c in range(NCORES) for w in range(W)] + [N]
    cut = np.searchsorted(d, np.asarray(bounds))
    nlo_cw = np.zeros((NCORES, W), np.int64)
    nhi_cw = np.zeros((NCORES, W), np.int64)
    for i in range(NCORES * W):
        sw = s[cut[i] : cut[i + 1]]
        nlo = int((sw < SPLIT).sum())
        nlo_cw[i // W, i % W] = nlo
        nhi_cw[i // W, i % W] = len(sw) - nlo
    return nlo_cw, nhi_cw


_CACHE = {}


def run_gcn(inputs, n_nodes, n_graphs, d_in=14, d_hid=256, d_fc=128, n_cls=2,
            grp=3, trace=False):
    nlo_cw, nhi_cw = compute_tile_budget(inputs, n_nodes, n_nodes // NCORES)
    cfg = Cfg(n_nodes, n_graphs, d_in, d_hid, d_fc, n_cls, nlo_cw, nhi_cw, grp)
    per_core, shared = host_prep(inputs, cfg)
    key = (n_nodes, n_graphs, tuple(cfg.T_LO), tuple(cfg.T_HI), grp, shared["has_b2"])
    if key not in _CACHE:
        _CACHE[key] = build_nc(cfg, shared["has_b2"])
    nc = _CACHE[key]
    res = run_bass_kernel_spmd(nc, per_core, list(range(NCORES)), trace=trace)
    return res.results[0]["out"].astype(np.float32), res


def kernel(**inputs) -> np.ndarray:
    out, _ = run_gcn(
        inputs, n_nodes=50000, n_graphs=256,
        trace=bool(int(os.environ.get("GCN_TRACE", "0"))),
    )
    return out



# revision 9
# speedup vs baseline: 1.6420x; 1.6420x over previous
"""Trainium2 Bass kernel for a 2-layer GCN + global mean pool + MLP head.

Strategy (8 NeuronCores, SPMD), v2:
  - Nodes (and incident edges grouped by destination window) sharded across
    8 cores; each core owns N/8 destination nodes.
  - Layer 1 needs NO dynamic gather at all: the gather indices are static
    (edge_index is a host input), so the host materializes the per-edge
    source features (x[src] * dinv[src] * dinv[dst], 14 cols padded to 16,
    bf16) directly in gather-slot layout.  The kernel streams the slab with
    large static DMAs and aggregates per destination window with one-hot
    matmuls, producing aggT = (A_norm x)^T with no transpose step.
  - Layer 2's table h2t = (h1 @ W2) * dinv is computed on-chip in fp8,
    AllGather-ed in two chunks (first chunk overlaps the second half of
    phase A), and gathered per edge with dma_gather spread over FOUR SWDGE
    queues (descriptor generation runs concurrently per queue on separate
    GpSimd core pairs).
  - selu(z) = lam*Relu(z) - lam*alpha*Relu(1 - exp(z)): 3 scalar acts +
    one DVE subtract.
  - Mean-pool partial sums via one-hot matmuls, AllReduce, tiny MLP head
    replicated on every core.
"""

import os
import numpy as np
import ml_dtypes

import concourse.bacc as bacc
import concourse.bass as bass
import concourse.mybir as mybir
import concourse.tile as tile
from concourse.bass_utils import run_bass_kernel_spmd

F32 = mybir.dt.float32
F8 = mybir.dt.float8e4
BF16 = mybir.dt.bfloat16
I16 = mybir.dt.int16
AF = mybir.ActivationFunctionType
OP = mybir.AluOpType
NPBF16 = ml_dtypes.bfloat16
NPF8 = ml_dtypes.float8_e4m3

SELU_LAM = 1.0507009873554805
SELU_ALPHA = 1.6732632423543772
SELU_LA = SELU_LAM * SELU_ALPHA

P = 128
NCORES = 8
NQ = 4  # SWDGE queues


class Cfg:
    def __init__(self, n_nodes, n_graphs, d_in, d_hid, d_fc, n_cls, tq_cw, grp):
        assert n_nodes % NCORES == 0
        self.N = n_nodes
        self.G = n_graphs
        self.DIN = d_in
        self.DH = d_hid          # 256
        self.DFC = d_fc          # 128
        self.NCLS = n_cls        # 2
        self.NSH = n_nodes // NCORES
        self.W = -(-self.NSH // P)          # dst windows per core (49)
        self.GRP = grp
        # h2 table split: chunk A = first WA windows (aligned to the group
        # grid so the chunked AllGather fires at a group boundary)
        self.WA = ((self.W // 2) // grp) * grp
        self.ROWSA = min(self.WA * P, self.NSH)
        self.ROWSB = self.NSH - self.ROWSA
        self.NA = NCORES * self.ROWSA       # rows in table A
        self.NB = NCORES * self.ROWSB       # rows in table B
        # queue split points (relative to table A / B)
        self.QA = -(-self.NA // 2)
        self.QB = -(-self.NB // 2)
        assert self.QA < 32768 and self.NA - self.QA < 32768
        assert self.QB < 32768 and self.NB - self.QB < 32768
        # groups of windows
        self.groups = []
        w = 0
        while w < self.W:
            wg = min(grp, self.W - w)
            self.groups.append((w, wg))
            w += wg
        # per-(group, queue) tile budgets: max over cores and windows in group
        self.TQ = []  # [g][q]
        for (w0, wg) in self.groups:
            tq = []
            for q in range(NQ):
                t = max(-(-int(tq_cw[c, w, q]) // P)
                        for c in range(NCORES) for w in range(w0, w0 + wg))
                tq.append(max(t, 1))
            self.TQ.append(tq)
        # slot layout: group g -> [q0 tiles window-major][q1 ...] ...
        self.grp_slot_base = []
        self.grp_q_off = []   # [g][q]: slot offset within group
        self.grp_idx_col = []  # [g][q]: idx slab col offset
        slot = 0
        col = 0
        for g, (w0, wg) in enumerate(self.groups):
            self.grp_slot_base.append(slot)
            qoffs = []
            icols = []
            off = 0
            for q in range(NQ):
                qoffs.append(off)
                icols.append(col)
                nq = wg * self.TQ[g][q]
                off += nq
                col += nq * 8          # (nq*128)/16 int16 cols
            self.grp_q_off.append(qoffs)
            self.grp_idx_col.append(icols)
            slot += off
        self.SLOT_TOT = slot
        self.IDX_COLS = col
        self.grp_nslots = [sum(wg * t for t in self.TQ[g])
                           for g, (w0, wg) in enumerate(self.groups)]


def _win_slots(cfg, g, k):
    """Global slot ids of window k of group g (all queues)."""
    base = cfg.grp_slot_base[g]
    out = []
    for q in range(NQ):
        t0 = base + cfg.grp_q_off[g][q] + k * cfg.TQ[g][q]
        out.extend(range(t0, t0 + cfg.TQ[g][q]))
    return out


def edge_queue_map(cfg, s):
    """Map source node ids -> (queue, relative idx) for the split h2 tables."""
    r = s // cfg.NSH
    i = s % cfg.NSH
    in_a = i < cfg.ROWSA
    idx_a = r * cfg.ROWSA + i
    idx_b = r * cfg.ROWSB + (i - cfg.ROWSA)
    q = np.where(in_a,
                 np.where(idx_a < cfg.QA, 0, 1),
                 np.where(idx_b < cfg.QB, 2, 3))
    rel = np.where(in_a,
                   np.where(idx_a < cfg.QA, idx_a, idx_a - cfg.QA),
                   np.where(idx_b < cfg.QB, idx_b, idx_b - cfg.QB))
    return q.astype(np.int64), rel.astype(np.int64)


def sort_edges(inputs, n_nodes):
    ei = np.asarray(inputs["edge_index"], np.int64)
    N = n_nodes
    src = np.concatenate([ei[0], np.arange(N, dtype=np.int64)])
    dst = np.concatenate([ei[1], np.arange(N, dtype=np.int64)])
    order = np.argsort(dst, kind="stable")
    return src[order], dst[order]


def compute_tile_budget(cfg_like, s, d, n_nodes, nsh):
    """Per-(core, window, queue) edge counts."""
    W = -(-nsh // P)
    bounds = [c * nsh + w * P for c in range(NCORES) for w in range(W)] + [n_nodes]
    cut = np.searchsorted(d, np.asarray(bounds))
    q, _ = edge_queue_map(cfg_like, s)
    tq_cw = np.zeros((NCORES, W, NQ), np.int64)
    for i in range(NCORES * W):
        qs = q[cut[i]: cut[i + 1]]
        for qq in range(NQ):
            tq_cw[i // W, i % W, qq] = int((qs == qq).sum())
    return tq_cw, cut


class CfgLike:
    """Just enough geometry for edge_queue_map before the full Cfg exists."""
    def __init__(self, n_nodes, grp):
        self.N = n_nodes
        self.NSH = n_nodes // NCORES
        self.W = -(-self.NSH // P)
        self.WA = ((self.W // 2) // grp) * grp
        self.ROWSA = min(self.WA * P, self.NSH)
        self.ROWSB = self.NSH - self.ROWSA
        self.NA = NCORES * self.ROWSA
        self.NB = NCORES * self.ROWSB
        self.QA = -(-self.NA // 2)
        self.QB = -(-self.NB // 2)


def host_prep(inputs, cfg, s, d, cut):
    N, G = cfg.N, cfg.G
    x = np.asarray(inputs["x"], np.float64)
    batch = np.asarray(inputs["batch"], np.int64)

    deg = np.bincount(d, minlength=N).astype(np.float64)
    dinv = 1.0 / np.sqrt(np.maximum(deg, 1.0))
    norm = dinv[s] * dinv[d]                       # per (sorted) edge
    qmap, rel = edge_queue_map(cfg, s)

    W = cfg.W
    cnt = np.bincount(batch, minlength=G).astype(np.float32)
    cntinv = (1.0 / np.maximum(cnt, 1.0)).astype(np.float32)

    per_core = []
    for c in range(NCORES):
        atab = np.zeros((P, cfg.SLOT_TOT, 16), NPBF16)
        oht = np.zeros((P, cfg.SLOT_TOT, P), NPF8)
        iflat = np.zeros((cfg.SLOT_TOT, P), np.int16)
        for g, (w0, wg) in enumerate(cfg.groups):
            for k in range(wg):
                w = w0 + k
                a, b = cut[c * W + w], cut[c * W + w + 1]
                sw, dw = s[a:b], d[a:b]
                nw, rw, qw = norm[a:b], rel[a:b], qmap[a:b]
                dl = (dw - (c * cfg.NSH + w * P)).astype(np.int64)
                for q in range(NQ):
                    m = qw == q
                    se, de, ne, re = sw[m], dl[m], nw[m], rw[m]
                    t0 = cfg.grp_slot_base[g] + cfg.grp_q_off[g][q] + k * cfg.TQ[g][q]
                    j = np.arange(len(se))
                    slot = t0 + j // P
                    lane = j % P
                    atab[lane, slot, : cfg.DIN] = (
                        x[se] * ne[:, None]).astype(NPBF16)
                    oht[lane, slot, de] = 1.0
                    iflat[slot, lane] = re.astype(np.int16)
        # idx slab: per (g, q) the stream is its contiguous slot range
        idx_slab = np.zeros((16, cfg.IDX_COLS), np.int16)
        for g, (w0, wg) in enumerate(cfg.groups):
            for q in range(NQ):
                t0 = cfg.grp_slot_base[g] + cfg.grp_q_off[g][q]
                nq = wg * cfg.TQ[g][q]
                stream = iflat[t0: t0 + nq].reshape(-1)
                col0 = cfg.grp_idx_col[g][q]
                idx_slab[:, col0: col0 + len(stream) // 16] = (
                    stream.reshape(-1, 16).T)

        dinv_w = np.zeros((P, W), np.float32)
        batchloc = np.full((P, W), -1.0, np.float32)
        base = c * cfg.NSH
        for w in range(W):
            rows = min(P, cfg.NSH - w * P)
            dinv_w[:rows, w] = dinv[base + w * P: base + w * P + rows]
            batchloc[:rows, w] = batch[base + w * P: base + w * P + rows]
        ohg = (batchloc[:, :, None] == np.arange(G, dtype=np.float32)[None, None, :])
        ohg = ohg.astype(NPBF16).reshape(P, W * G)

        per_core.append({
            "atab": atab.reshape(P, cfg.SLOT_TOT * 16),
            "idxs": np.tile(idx_slab, (8, 1)),
            "oht": oht.reshape(P, cfg.SLOT_TOT * P),
            "ohgt": ohg,
            "dinv_w": dinv_w,
        })

    # shared constants
    W1p = np.zeros((16, cfg.DH), NPBF16)
    W1p[: cfg.DIN] = np.asarray(inputs["W1"], np.float32).astype(NPBF16)
    W2 = np.asarray(inputs["W2"], np.float32)
    W2_sb = np.concatenate([W2[:P, :], W2[P:, :]], axis=1).astype(NPBF16)
    b1 = np.asarray(inputs["b1"], np.float32).reshape(2, P).T.copy()  # [128,2]
    b2 = np.asarray(inputs["b2"], np.float32)
    b2b = np.tile(b2[None, :], (P, 1))
    fc1 = np.asarray(inputs["fc1_w"], np.float32)
    fc1_sb = np.concatenate([fc1[:P, :], fc1[P:, :]], axis=1)  # [128, 256]
    fc1b = np.asarray(inputs["fc1_b"], np.float32).reshape(P, 1)
    fc2 = np.asarray(inputs["fc2_w"], np.float32)  # [128, 2]
    fc2b = np.zeros((P, 1), np.float32)
    fc2b[: cfg.NCLS, 0] = np.asarray(inputs["fc2_b"], np.float32)
    ident = np.eye(P, dtype=np.float32)
    cntinv2 = np.tile(cntinv[None, :], (P, 2))  # [128, 2*G]

    shared = {
        "W1p": W1p,
        "W2_sb": W2_sb,
        "b1h": b1,
        "b2b": b2b.astype(np.float32),
        "fc1_sb": fc1_sb,
        "fc1b": fc1b,
        "fc2_sb": fc2,
        "fc2b": fc2b,
        "ident": ident,
        "cntinv2": cntinv2,
        "has_b2": bool(np.any(b2b)),
    }
    for im in per_core:
        for k, v in shared.items():
            if k != "has_b2":
                im[k] = v
    return per_core, shared


def build_nc(cfg, has_b2):
    nc = bacc.Bacc("TRN2", target_bir_lowering=False, debug=False,
                   num_devices=NCORES, num_swdge_queues=NQ)
    N, G, W = cfg.N, cfg.G, cfg.W
    DH = cfg.DH

    atab_d = nc.dram_tensor("atab", [P, cfg.SLOT_TOT * 16], BF16, kind="ExternalInput")
    idxs = nc.dram_tensor("idxs", [P, cfg.IDX_COLS], I16, kind="ExternalInput")
    oht_d = nc.dram_tensor("oht", [P, cfg.SLOT_TOT * P], F8, kind="ExternalInput")
    dinv_d = nc.dram_tensor("dinv_w", [P, W], F32, kind="ExternalInput")
    ohgt_d = nc.dram_tensor("ohgt", [P, W * G], BF16, kind="ExternalInput")
    W1p_d = nc.dram_tensor("W1p", [16, DH], BF16, kind="ExternalInput")
    W2_d = nc.dram_tensor("W2_sb", [P, 2 * DH], BF16, kind="ExternalInput")
    b1_d = nc.dram_tensor("b1h", [P, 2], F32, kind="ExternalInput")
    b2b_d = nc.dram_tensor("b2b", [P, DH], F32, kind="ExternalInput")
    fc1_d = nc.dram_tensor("fc1_sb", [P, 2 * cfg.DFC], F32, kind="ExternalInput")
    fc1b_d = nc.dram_tensor("fc1b", [P, 1], F32, kind="ExternalInput")
    fc2_d = nc.dram_tensor("fc2_sb", [cfg.DFC, cfg.NCLS], F32, kind="ExternalInput")
    fc2b_d = nc.dram_tensor("fc2b", [P, 1], F32, kind="ExternalInput")
    ident_d = nc.dram_tensor("ident", [P, P], F32, kind="ExternalInput")
    cntinv2_d = nc.dram_tensor("cntinv2", [P, 2 * G], F32, kind="ExternalInput")

    out_d = nc.dram_tensor("out", [G, cfg.NCLS], F32, kind="ExternalOutput")

    shard_a = nc.dram_tensor("shard_a", [cfg.ROWSA, DH], F8)
    shard_b = nc.dram_tensor("shard_b", [cfg.ROWSB, DH], F8)
    h2ta = nc.dram_tensor("h2ta", [cfg.NA, DH], F8, addr_space="Shared")
    h2tb = nc.dram_tensor("h2tb", [cfg.NB, DH], F8, addr_space="Shared")
    pool_part = nc.dram_tensor("pool_part", [2 * P, G], F32)
    pool_sum = nc.dram_tensor("pool_sum", [2 * P, G], F32, addr_space="Shared")

    la_tile = [None]

    def selu3(scal, vec, out_ap, z_ap, tmp_pool, shape, bias=None):
        """out = selu(z [+ bias]) = lam*Relu(z+b) - lam*a*Relu(1 - exp(z+b))."""
        r1 = tmp_pool.tile(shape, F32, tag="selu_r1")
        e = tmp_pool.tile(shape, F32, tag="selu_e")
        r2 = tmp_pool.tile(shape, F32, tag="selu_r2")
        if bias is None:
            scal.activation(r1[:], z_ap, AF.Relu, scale=SELU_LAM)
            scal.activation(e[:], z_ap, AF.Exp)
        else:
            # bias given as (bias_ap, lam_bias_ap) per-partition columns
            b_ap, lb_ap = bias
            scal.activation(r1[:], z_ap, AF.Relu, bias=lb_ap, scale=SELU_LAM)
            scal.activation(e[:], z_ap, AF.Exp, bias=b_ap)
        scal.activation(r2[:], e[:], AF.Relu, bias=la_tile[0][:, 0:1],
                        scale=-SELU_LA)
        vec.tensor_tensor(out_ap, r1[:], r2[:], OP.subtract)

    with tile.TileContext(nc) as tc:
        with (
            tc.tile_pool(name="consts", bufs=1) as cpool,
            tc.tile_pool(name="idxpool", bufs=1) as ipool,
            tc.tile_pool(name="atab", bufs=2) as apool,
            tc.tile_pool(name="gx2", bufs=2) as gx2pool,
            tc.tile_pool(name="oh", bufs=2) as ohpool,
            tc.tile_pool(name="work", bufs=3) as wpool,
            tc.tile_pool(name="head", bufs=1) as hpool,
            tc.tile_pool(name="post", bufs=2) as ppool,
            tc.tile_pool(name="ps_sm", bufs=2, space="PSUM") as ps_sm,
            tc.tile_pool(name="ps_h1", bufs=2, space="PSUM") as ps_h1,
            tc.tile_pool(name="ps_h2", bufs=2, space="PSUM") as ps_h2,
            tc.tile_pool(name="ps_pool", bufs=1, space="PSUM") as ps_pool,
        ):
            def load(pool, dram, shape, dt):
                t = pool.tile(shape, dt, tag=dram.name + "_sb")
                nc.sync.dma_start(out=t[:], in_=dram[tuple(slice(0, s) for s in shape)])
                return t

            la = cpool.tile([P, 1], F32, tag="la_const")
            nc.vector.memset(la[:], SELU_LA)
            la_tile[0] = la

            idx_sb = load(ipool, idxs, [P, cfg.IDX_COLS], I16)
            dinv_sb = load(cpool, dinv_d, [P, W], F32)
            W1p_sb = load(cpool, W1p_d, [16, DH], BF16)
            W2_sb = load(cpool, W2_d, [P, 2 * DH], BF16)
            b1_sb = load(cpool, b1_d, [P, 2], F32)
            b1l_sb = cpool.tile([P, 2], F32, tag="b1l")
            nc.scalar.activation(b1l_sb[:], b1_sb[:], AF.Copy, scale=SELU_LAM)
            b2b_sb = load(cpool, b2b_d, [P, DH], F32) if has_b2 else None
            fc1_sb = load(cpool, fc1_d, [P, 2 * cfg.DFC], F32)
            fc1b_sb = load(cpool, fc1b_d, [P, 1], F32)
            fc1bl_sb = cpool.tile([P, 1], F32, tag="fc1bl")
            nc.scalar.activation(fc1bl_sb[:], fc1b_sb[:], AF.Copy, scale=SELU_LAM)
            fc2_sb = load(cpool, fc2_d, [cfg.DFC, cfg.NCLS], F32)
            fc2b_sb = load(cpool, fc2b_d, [P, 1], F32)
            ident_sb = load(cpool, ident_d, [P, P], F32)
            cntinv2_sb = load(cpool, cntinv2_d, [P, 2 * G], F32)

            def load_onehots(g):
                base = cfg.grp_slot_base[g]
                ns = cfg.grp_nslots[g]
                ohsl = ohpool.tile([P, ns * P], F8, tag="ohslab")
                nc.sync.dma_start(
                    out=ohsl[:], in_=oht_d[:, base * P: (base + ns) * P])
                return ohsl

            # ================= Phase A: layer 1 -> shard_a / shard_b ========
            for g, (w0, wg) in enumerate(cfg.groups):
                base = cfg.grp_slot_base[g]
                ns = cfg.grp_nslots[g]
                at = apool.tile([P, ns * 16], BF16, tag="atab_t")
                nc.sync.dma_start(out=at[:], in_=atab_d[:, base * 16: (base + ns) * 16])
                ohsl = load_onehots(g)
                for k in range(wg):
                    w = w0 + k
                    slots = _win_slots(cfg, g, k)
                    psA = ps_sm.tile([16, P], F32, tag="sm")
                    nslot = len(slots)
                    for t, sl in enumerate(slots):
                        sloc = sl - base
                        nc.tensor.matmul(
                            psA[:], at[:, sloc * 16: sloc * 16 + 16],
                            ohsl[:, sloc * P: (sloc + 1) * P],
                            start=(t == 0), stop=(t == nslot - 1),
                        )
                    aggT = wpool.tile([16, P], BF16, tag="aggT")
                    nc.scalar.copy(aggT[:], psA[:])
                    ph1 = ps_h1.tile([P, DH], F32, tag="ph1")
                    for j in range(2):
                        nc.tensor.matmul(
                            ph1[:, j * P: (j + 1) * P],
                            W1p_sb[:, j * P: (j + 1) * P], aggT[:],
                            start=True, stop=True,
                        )
                    # selu with per-half bias
                    r1 = ppool.tile([P, DH], F32, tag="a_r1")
                    e = ppool.tile([P, DH], F32, tag="a_e")
                    r2 = ppool.tile([P, DH], F32, tag="a_r2")
                    for j in range(2):
                        sl_ = slice(j * P, (j + 1) * P)
                        nc.scalar.activation(r1[:, sl_], ph1[:, sl_], AF.Relu,
                                             bias=b1l_sb[:, j: j + 1], scale=SELU_LAM)
                        nc.scalar.activation(e[:, sl_], ph1[:, sl_], AF.Exp,
                                             bias=b1_sb[:, j: j + 1])
                    nc.scalar.activation(r2[:], e[:], AF.Relu,
                                         bias=la_tile[0][:, 0:1], scale=-SELU_LA)
                    h1T = ppool.tile([P, DH], BF16, tag="a_h1T")
                    nc.vector.tensor_tensor(h1T[:], r1[:], r2[:], OP.subtract)

                    psum_h2t = ps_h2.tile([P, DH], F32, tag="main")
                    for j in range(2):
                        nc.tensor.matmul(
                            psum_h2t[:], h1T[:, j * P: (j + 1) * P],
                            W2_sb[:, j * DH: (j + 1) * DH],
                            start=(j == 0), stop=(j == 1),
                        )
                    h2tw = ppool.tile([P, DH], F8, tag="h2tw")
                    nc.scalar.activation(h2tw[:], psum_h2t[:], AF.Copy,
                                         scale=dinv_sb[:, w: w + 1])
                    rows = min(P, cfg.NSH - w * P)
                    if w < cfg.WA:
                        nc.sync.dma_start(out=shard_a[w * P: w * P + rows, :],
                                          in_=h2tw[:rows, :])
                    else:
                        r0 = w * P - cfg.ROWSA
                        nc.sync.dma_start(out=shard_b[r0: r0 + rows, :],
                                          in_=h2tw[:rows, :])
                # chunked AllGather: fire A as soon as its windows are done
                if w0 + wg == cfg.WA:
                    nc.gpsimd.collective_compute(
                        "AllGather", OP.bypass,
                        replica_groups=[list(range(NCORES))],
                        ins=[shard_a[:, :]], outs=[h2ta[:, :]],
                    )
            nc.gpsimd.collective_compute(
                "AllGather", OP.bypass,
                replica_groups=[list(range(NCORES))],
                ins=[shard_b[:, :]], outs=[h2tb[:, :]],
            )

            # ================= Phase B: layer 2 + pooling ===================
            pp0 = ps_pool.tile([P, G], F32, tag="pp0")
            pp1 = ps_pool.tile([P, G], F32, tag="pp1")
            pps = [pp0, pp1]
            qsrc = [
                (h2ta, 0, cfg.QA), (h2ta, cfg.QA, cfg.NA),
                (h2tb, 0, cfg.QB), (h2tb, cfg.QB, cfg.NB),
            ]
            for g, (w0, wg) in enumerate(cfg.groups):
                base = cfg.grp_slot_base[g]
                ns = cfg.grp_nslots[g]
                gt2 = gx2pool.tile([P, ns, DH], F8, tag="gx2_t")
                for q in range(NQ):
                    tab, lo, hi = qsrc[q]
                    nq = wg * cfg.TQ[g][q]
                    s0 = cfg.grp_q_off[g][q]
                    nc.gpsimd.dma_gather(
                        gt2[:, s0: s0 + nq, :],
                        tab[lo: hi, :],
                        idx_sb[:, cfg.grp_idx_col[g][q]:
                               cfg.grp_idx_col[g][q] + nq * 8],
                        nq * P, nq * P, DH,
                        single_packet=False, queue_num=q,
                    )
                ohsl = load_onehots(g)
                ohg_sl = ohpool.tile([P, wg * G], BF16, tag="ohg_slab")
                nc.sync.dma_start(out=ohg_sl[:], in_=ohgt_d[:, w0 * G: (w0 + wg) * G])
                for k in range(wg):
                    w = w0 + k
                    slots = _win_slots(cfg, g, k)
                    psum2 = ps_h2.tile([P, DH], F32, tag="main")
                    nslot = len(slots)
                    for t, sl in enumerate(slots):
                        sloc = sl - base
                        nc.tensor.matmul(
                            psum2[:], ohsl[:, sloc * P: (sloc + 1) * P],
                            gt2[:, sloc, :],
                            start=(t == 0), stop=(t == nslot - 1),
                        )
                    zd = ppool.tile([P, DH], F32, tag="b_zd")
                    nc.scalar.activation(zd[:], psum2[:], AF.Copy,
                                         scale=dinv_sb[:, w: w + 1])
                    if has_b2:
                        zb2 = ppool.tile([P, DH], F32, tag="b_zb2")
                        nc.vector.tensor_tensor(zb2[:], zd[:], b2b_sb[:], OP.add)
                        zd = zb2
                    h2w = ppool.tile([P, DH], BF16, tag="b_h2w")
                    selu3(nc.scalar, nc.vector, h2w[:], zd[:], ppool, [P, DH])
                    for j in range(2):
                        nc.tensor.matmul(
                            pps[j][:], h2w[:, j * P: (j + 1) * P],
                            ohg_sl[:, k * G: (k + 1) * G],
                            start=(w == 0), stop=(w == W - 1),
                        )

            # ================= pooled head =================
            pT = hpool.tile([P, 2 * G], F32, tag="pT")
            nc.scalar.copy(pT[:, 0:G], pp0[:])
            nc.scalar.copy(pT[:, G: 2 * G], pp1[:])
            nc.sync.dma_start(out=pool_part[0:P, :], in_=pT[:, 0:G])
            nc.sync.dma_start(out=pool_part[P: 2 * P, :], in_=pT[:, G: 2 * G])
            nc.gpsimd.collective_compute(
                "AllReduce", OP.add,
                replica_groups=[list(range(NCORES))],
                ins=[pool_part[:, :]], outs=[pool_sum[:, :]],
            )
            ps = hpool.tile([P, 2 * G], F32, tag="ps_in")
            nc.sync.dma_start(out=ps[:, 0:G], in_=pool_sum[0:P, :])
            nc.sync.dma_start(out=ps[:, G: 2 * G], in_=pool_sum[P: 2 * P, :])
            pm = hpool.tile([P, 2 * G], F32, tag="pm")
            nc.vector.tensor_tensor(pm[:], ps[:], cntinv2_sb[:], OP.mult)
            gsel = hpool.tile([P, 2 * G], F32, tag="gsel")
            selu3(nc.scalar, nc.vector, gsel[:], pm[:], hpool, [P, 2 * G])

            psum_fc1 = ps_h2.tile([P, G], F32, tag="main")
            for j in range(2):
                nc.tensor.matmul(
                    psum_fc1[:], fc1_sb[:, j * cfg.DFC: (j + 1) * cfg.DFC],
                    gsel[:, j * G: (j + 1) * G],
                    start=(j == 0), stop=(j == 1),
                )
            hsel = hpool.tile([P, G], F32, tag="hsel")
            selu3(nc.scalar, nc.vector, hsel[:], psum_fc1[:], hpool, [P, G],
                  bias=(fc1b_sb[:, 0:1], fc1bl_sb[:, 0:1]))

            psum_fc2 = ps_sm.tile([cfg.NCLS, G], F32, tag="sm")
            nc.tensor.matmul(psum_fc2[:], fc2_sb[:], hsel[:], start=True, stop=True)
            lg2 = wpool.tile([cfg.NCLS, G], F32, tag="lg2")
            nc.scalar.activation(lg2[:], psum_fc2[:], AF.Identity,
                                 bias=fc2b_sb[0: cfg.NCLS, 0:1])
            for j in range(-(-G // P)):
                gw = min(P, G - j * P)
                psT2 = ps_sm.tile([P, cfg.NCLS], F32, tag="sm")
                nc.tensor.transpose(
                    psT2[:gw, :], lg2[:, j * P: j * P + gw],
                    ident_sb[0: cfg.NCLS, 0: cfg.NCLS],
                )
                lgj = hpool.tile([P, cfg.NCLS], F32, tag="lgj")
                nc.scalar.copy(lgj[:gw, :], psT2[:gw, :])
                nm = hpool.tile([P, 1], F32, tag="nm")
                nc.vector.tensor_reduce(
                    nm[:gw, :], lgj[:gw, :], mybir.AxisListType.X, OP.max,
                    negate=True)
                e4 = hpool.tile([P, cfg.NCLS], F32, tag="e4")
                nc.scalar.activation(e4[:gw, :], lgj[:gw, :], AF.Exp,
                                     bias=nm[:gw, 0:1])
                s4 = hpool.tile([P, 1], F32, tag="s4")
                nc.vector.tensor_reduce(s4[:gw, :], e4[:gw, :],
                                        mybir.AxisListType.X, OP.add)
                ls = hpool.tile([P, 1], F32, tag="ls")
                nc.scalar.activation(ls[:gw, :], s4[:gw, :], AF.Ln)
                q_ = hpool.tile([P, 1], F32, tag="q")
                nc.vector.tensor_tensor(q_[:gw, :], nm[:gw, :], ls[:gw, :],
                                        OP.subtract)
                outj = hpool.tile([P, cfg.NCLS], F32, tag="outj")
                nc.vector.tensor_scalar(outj[:gw, :], lgj[:gw, :],
                                        q_[:gw, 0:1], None, OP.add)
                nc.sync.dma_start(out=out_d[j * P: j * P + gw, :],
                                  in_=outj[:gw, :])

    nc.compile()
    return nc


_CACHE = {}


def run_gcn(inputs, n_nodes, n_graphs, d_in=14, d_hid=256, d_fc=128, n_cls=2,
            grp=3, trace=False):
    cl = CfgLike(n_nodes, grp)
    s, d = sort_edges(inputs, n_nodes)
    tq_cw, cut = compute_tile_budget(cl, s, d, n_nodes, n_nodes // NCORES)
    cfg = Cfg(n_nodes, n_graphs, d_in, d_hid, d_fc, n_cls, tq_cw, grp)
    per_core, shared = host_prep(inputs, cfg, s, d, cut)
    key = (n_nodes, n_graphs, grp, shared["has_b2"],
           tuple(tuple(t) for t in cfg.TQ))
    if key not in _CACHE:
        _CACHE[key] = build_nc(cfg, shared["has_b2"])
    nc = _CACHE[key]
    res = run_bass_kernel_spmd(nc, per_core, list(range(NCORES)), trace=trace)
    return res.results[0]["out"].astype(np.float32), res


def kernel(**inputs) -> np.ndarray:
    out, _ = run_gcn(
        inputs, n_nodes=50000, n_graphs=256,
        trace=bool(int(os.environ.get("GCN_TRACE", "0"))),
    )
    return out


# revision 14
# speedup vs baseline: 2.3388x; 1.4244x over previous
"""Trainium2 Bass kernel for a 2-layer GCN + global mean pool + MLP head.

Strategy (8 NeuronCores, SPMD), v3:
  - Nodes (and incident edges grouped by destination window) sharded across
    8 cores; each core owns N/8 destination nodes.
  - Layer 1 needs NO dynamic gather: gather indices are static (edge_index
    is a host input), so the host materializes per-edge source features
    (x[src] * dinv[src] * dinv[dst], 14 cols padded to 16, fp8) directly in
    gather-slot layout.  The kernel streams the slab with large static DMAs
    and aggregates per destination window with one-hot DoubleRow-fp8
    matmuls, producing aggT = (A_norm x)^T with no transpose step.
  - Layer 2's table h2t = (h1 @ W2) * dinv is computed on-chip in fp8 and
    AllGather-ed in two chunks (A = first ~half of windows overlaps the
    rest of phase A).  Each chunk table has < 32768 rows so int16 gather
    indices address it with a single base.  Per-edge dma_gather spreads
    over FOUR SWDGE queues (two per chunk, tile-granular balanced split;
    descriptor generation runs concurrently per queue on separate GpSimd
    core pairs).
  - selu(z) = lam*Relu(z) - lam*alpha*Relu(1 - exp(z)): 3 scalar acts +
    one DVE subtract.
  - Mean-pool partials via one-hot matmuls with graphs on the partition
    dim, ReduceScatter, per-core MLP head on G/8 graphs; the host
    assembles the output slices.
"""

import os
import numpy as np
import ml_dtypes

import concourse.bacc as bacc
import concourse.bass as bass
import concourse.mybir as mybir
import concourse.tile as tile
from concourse.bass_utils import run_bass_kernel_spmd

F32 = mybir.dt.float32
F8 = mybir.dt.float8e4
BF16 = mybir.dt.bfloat16
I16 = mybir.dt.int16
AF = mybir.ActivationFunctionType
OP = mybir.AluOpType
DR = mybir.MatmulPerfMode.DoubleRow
NPBF16 = ml_dtypes.bfloat16
NPF8 = ml_dtypes.float8_e4m3

SELU_LAM = 1.0507009873554805
SELU_ALPHA = 1.6732632423543772
SELU_LA = SELU_LAM * SELU_ALPHA

P = 128
NCORES = 8
NQ = 4  # SWDGE queues


def share_of(T, k, qpar):
    """Window-parity-alternating split of T tiles between the 2 queues of a
    table half: queue parity 0 gets ceil on even windows, floor on odd."""
    hi, lo = -(-T // 2), T // 2
    if k % 2 == 0:
        return hi if qpar == 0 else lo
    return lo if qpar == 0 else hi


class Cfg:
    def __init__(self, n_nodes, n_graphs, d_in, d_hid, d_fc, n_cls, th_cw, grp):
        assert n_nodes % NCORES == 0
        self.N = n_nodes
        self.G = n_graphs
        self.DIN = d_in
        self.DH = d_hid          # 256
        self.DFC = d_fc          # 128
        self.NCLS = n_cls        # 2
        self.NSH = n_nodes // NCORES
        self.W = -(-self.NSH // P)          # dst windows per core (49)
        self.GRP = grp
        self.GSH = n_graphs // NCORES       # graphs per core for the head
        # h2 table split: chunk A = first WA windows (aligned to the group
        # grid so the chunked AllGather fires at a group boundary)
        self.WA = ((self.W // 2) // grp) * grp
        self.ROWSA = min(self.WA * P, self.NSH)
        self.ROWSB = self.NSH - self.ROWSA
        self.NA = NCORES * self.ROWSA       # rows in table A
        self.NB = NCORES * self.ROWSB       # rows in table B
        assert self.NA < 32768 and self.NB < 32768
        # groups of windows
        self.groups = []
        w = 0
        while w < self.W:
            wg = min(grp, self.W - w)
            self.groups.append((w, wg))
            w += wg
        # per-(group, half) tile budgets: max over cores and windows in group
        self.TH = []  # [g][h]
        for (w0, wg) in self.groups:
            th = []
            for h in range(2):
                t = max(-(-int(th_cw[c, w, h]) // P)
                        for c in range(NCORES) for w in range(w0, w0 + wg))
                th.append(max(t, 1))
            self.TH.append(th)
        # queue q -> (half, parity): q0=(A,0) q1=(A,1) q2=(B,0) q3=(B,1)
        # slot layout per group: [q0 run][q1 run][q2 run][q3 run],
        # each run is window-major with per-window share share_of(TH, k, par).
        self.grp_slot_base = []
        self.grp_q_off = []    # [g][q] slot offset within group
        self.grp_q_n = []      # [g][q] number of slots in run
        self.grp_idx_col = []  # [g][q] idx slab col offset
        # per (g, q, k): slot offset of window k's share within the group
        self.win_q_off = []    # [g][q][k]
        slot = 0
        col = 0
        for g, (w0, wg) in enumerate(self.groups):
            self.grp_slot_base.append(slot)
            qoffs, qns, icols, woffs = [], [], [], []
            off = 0
            for q in range(NQ):
                h, par = q // 2, q % 2
                qoffs.append(off)
                icols.append(col)
                wo = []
                for k in range(wg):
                    wo.append(off)
                    off += share_of(self.TH[g][h], k, par)
                woffs.append(wo)
                nq = off - qoffs[-1]
                qns.append(nq)
                col += nq * 8          # (nq*128)/16 int16 cols
            self.grp_q_off.append(qoffs)
            self.grp_q_n.append(qns)
            self.grp_idx_col.append(icols)
            self.win_q_off.append(woffs)
            slot += off
        self.SLOT_TOT = slot
        self.IDX_COLS = col
        self.grp_nslots = [sum(self.grp_q_n[g]) for g in range(len(self.groups))]


def _win_runs(cfg, g, k):
    """Per queue: (slot_offset_in_group, n_slots) of window k's share."""
    runs = []
    for q in range(NQ):
        h, par = q // 2, q % 2
        n = share_of(cfg.TH[g][h], k, par)
        runs.append((cfg.win_q_off[g][q][k], n))
    return runs


def edge_half_map(cfg, s):
    """src node id -> (half, table-local row)."""
    r = s // cfg.NSH
    i = s % cfg.NSH
    in_a = i < cfg.ROWSA
    h = np.where(in_a, 0, 1).astype(np.int64)
    rel = np.where(in_a, r * cfg.ROWSA + i,
                   r * cfg.ROWSB + (i - cfg.ROWSA)).astype(np.int64)
    return h, rel


def sort_edges(inputs, n_nodes):
    ei = np.asarray(inputs["edge_index"], np.int64)
    N = n_nodes
    src = np.concatenate([ei[0], np.arange(N, dtype=np.int64)])
    dst = np.concatenate([ei[1], np.arange(N, dtype=np.int64)])
    order = np.argsort(dst, kind="stable")
    return src[order], dst[order]


def compute_tile_budget(cfg_like, s, d, n_nodes, nsh):
    """Per-(core, window, half) edge counts."""
    W = -(-nsh // P)
    bounds = [c * nsh + w * P for c in range(NCORES) for w in range(W)] + [n_nodes]
    cut = np.searchsorted(d, np.asarray(bounds))
    h, _ = edge_half_map(cfg_like, s)
    th_cw = np.zeros((NCORES, W, 2), np.int64)
    for i in range(NCORES * W):
        hs = h[cut[i]: cut[i + 1]]
        th_cw[i // W, i % W, 0] = int((hs == 0).sum())
        th_cw[i // W, i % W, 1] = int((hs == 1).sum())
    return th_cw, cut


class CfgLike:
    def __init__(self, n_nodes, grp):
        self.N = n_nodes
        self.NSH = n_nodes // NCORES
        self.W = -(-self.NSH // P)
        self.WA = ((self.W // 2) // grp) * grp
        self.ROWSA = min(self.WA * P, self.NSH)
        self.ROWSB = self.NSH - self.ROWSA


def host_prep(inputs, cfg, s, d, cut):
    N, G = cfg.N, cfg.G
    x = np.asarray(inputs["x"], np.float64)
    batch = np.asarray(inputs["batch"], np.int64)

    deg = np.bincount(d, minlength=N).astype(np.float64)
    dinv = 1.0 / np.sqrt(np.maximum(deg, 1.0))
    norm = dinv[s] * dinv[d]
    hmap, rel = edge_half_map(cfg, s)

    W = cfg.W
    cnt = np.bincount(batch, minlength=G).astype(np.float32)
    cntinv = (1.0 / np.maximum(cnt, 1.0)).astype(np.float32)

    per_core = []
    for c in range(NCORES):
        atab = np.zeros((P, cfg.SLOT_TOT, 16), NPF8)
        oht = np.zeros((P, cfg.SLOT_TOT, P), NPF8)
        iflat = np.zeros((cfg.SLOT_TOT, P), np.int16)
        for g, (w0, wg) in enumerate(cfg.groups):
            base = cfg.grp_slot_base[g]
            for k in range(wg):
                w = w0 + k
                a, b = cut[c * W + w], cut[c * W + w + 1]
                sw, dw = s[a:b], d[a:b]
                nw, rw, hw = norm[a:b], rel[a:b], hmap[a:b]
                dl = (dw - (c * cfg.NSH + w * P)).astype(np.int64)
                for h in range(2):
                    m = hw == h
                    se, de, ne, re = sw[m], dl[m], nw[m], rw[m]
                    n = len(se)
                    # split positions between the two queues of this half
                    s0 = share_of(cfg.TH[g][h], k, 0) * P
                    for par in range(2):
                        q = h * 2 + par
                        t0 = base + cfg.win_q_off[g][q][k]
                        if par == 0:
                            seq = slice(0, min(n, s0))
                        else:
                            seq = slice(min(n, s0), n)
                        sq, dq, nq_, rq = se[seq], de[seq], ne[seq], re[seq]
                        j = np.arange(len(sq))
                        slotq = t0 + j // P
                        lane = j % P
                        atab[lane, slotq, : cfg.DIN] = (
                            x[sq] * nq_[:, None]).astype(NPF8)
                        oht[lane, slotq, dq] = 1.0
                        iflat[slotq, lane] = rq.astype(np.int16)
        idx_slab = np.zeros((16, cfg.IDX_COLS), np.int16)
        for g in range(len(cfg.groups)):
            for q in range(NQ):
                t0 = cfg.grp_slot_base[g] + cfg.grp_q_off[g][q]
                nq = cfg.grp_q_n[g][q]
                stream = iflat[t0: t0 + nq].reshape(-1)
                col0 = cfg.grp_idx_col[g][q]
                idx_slab[:, col0: col0 + len(stream) // 16] = (
                    stream.reshape(-1, 16).T)

        dinv_w = np.zeros((P, W), np.float32)
        batchloc = np.full((P, W), -1.0, np.float32)
        base_n = c * cfg.NSH
        for w in range(W):
            rows = min(P, cfg.NSH - w * P)
            dinv_w[:rows, w] = dinv[base_n + w * P: base_n + w * P + rows]
            batchloc[:rows, w] = batch[base_n + w * P: base_n + w * P + rows]
        ohg = (batchloc[:, :, None] == np.arange(G, dtype=np.float32)[None, None, :])
        ohg = ohg.astype(NPBF16).reshape(P, W * G)

        per_core.append({
            "atab": atab.reshape(P, cfg.SLOT_TOT * 16),
            "idxs": np.tile(idx_slab, (8, 1)),
            "oht": oht.reshape(P, cfg.SLOT_TOT * P),
            "ohgt": ohg,
            "dinv_w": dinv_w,
            "cntinv_sl": cntinv[c * cfg.GSH: (c + 1) * cfg.GSH].reshape(-1, 1),
        })

    # shared constants
    W1p = np.zeros((16, cfg.DH), NPBF16)
    W1p[: cfg.DIN] = np.asarray(inputs["W1"], np.float32).astype(NPBF16)
    W2 = np.asarray(inputs["W2"], np.float32)
    W2_sb = np.concatenate([W2[:P, :], W2[P:, :]], axis=1).astype(NPBF16)
    b1 = np.asarray(inputs["b1"], np.float32).reshape(2, P).T.copy()  # [128,2]
    b2 = np.asarray(inputs["b2"], np.float32)
    b2b = np.tile(b2[None, :], (P, 1)).astype(np.float32)
    fc1 = np.asarray(inputs["fc1_w"], np.float32)
    fc1_sb = np.concatenate([fc1[:P, :], fc1[P:, :]], axis=1)  # [128, 256]
    fc1b_bc = np.tile(np.asarray(inputs["fc1_b"], np.float32)[None, :],
                      (cfg.GSH, 1))                       # [GSH, 128]
    fc2 = np.asarray(inputs["fc2_w"], np.float32)         # [128, 2]
    fc2b_bc = np.tile(np.asarray(inputs["fc2_b"], np.float32)[None, :],
                      (cfg.GSH, 1))                       # [GSH, 2]
    ident = np.eye(P, dtype=np.float32)

    shared = {
        "W1p": W1p,
        "W2_sb": W2_sb,
        "b1h": b1,
        "b2b": b2b,
        "fc1_sb": fc1_sb,
        "fc1b_bc": fc1b_bc,
        "fc2_sb": fc2,
        "fc2b_bc": fc2b_bc,
        "ident": ident,
        "has_b1": bool(np.any(b1)),
        "has_b2": bool(np.any(b2b)),
    }
    for im in per_core:
        for k, v in shared.items():
            if k not in ("has_b1", "has_b2"):
                im[k] = v
    return per_core, shared


def build_nc(cfg, has_b1, has_b2):
    nc = bacc.Bacc("TRN2", target_bir_lowering=False, debug=False,
                   num_devices=NCORES, num_swdge_queues=NQ)
    N, G, W = cfg.N, cfg.G, cfg.W
    DH, GSH = cfg.DH, cfg.GSH

    atab_d = nc.dram_tensor("atab", [P, cfg.SLOT_TOT * 16], F8, kind="ExternalInput")
    idxs = nc.dram_tensor("idxs", [P, cfg.IDX_COLS], I16, kind="ExternalInput")
    oht_d = nc.dram_tensor("oht", [P, cfg.SLOT_TOT * P], F8, kind="ExternalInput")
    dinv_d = nc.dram_tensor("dinv_w", [P, W], F32, kind="ExternalInput")
    ohgt_d = nc.dram_tensor("ohgt", [P, W * G], BF16, kind="ExternalInput")
    W1p_d = nc.dram_tensor("W1p", [16, DH], BF16, kind="ExternalInput")
    W2_d = nc.dram_tensor("W2_sb", [P, 2 * DH], BF16, kind="ExternalInput")
    b1_d = nc.dram_tensor("b1h", [P, 2], F32, kind="ExternalInput")
    b2b_d = nc.dram_tensor("b2b", [P, DH], F32, kind="ExternalInput")
    fc1_d = nc.dram_tensor("fc1_sb", [P, 2 * cfg.DFC], F32, kind="ExternalInput")
    fc1b_d = nc.dram_tensor("fc1b_bc", [GSH, cfg.DFC], F32, kind="ExternalInput")
    fc2_d = nc.dram_tensor("fc2_sb", [cfg.DFC, cfg.NCLS], F32, kind="ExternalInput")
    fc2b_d = nc.dram_tensor("fc2b_bc", [GSH, cfg.NCLS], F32, kind="ExternalInput")
    ident_d = nc.dram_tensor("ident", [P, P], F32, kind="ExternalInput")
    cntinv_d = nc.dram_tensor("cntinv_sl", [GSH, 1], F32, kind="ExternalInput")

    out_d = nc.dram_tensor("out", [G, cfg.NCLS], F32, kind="ExternalOutput")

    shard_a = nc.dram_tensor("shard_a", [cfg.ROWSA, DH], F8)
    shard_b = nc.dram_tensor("shard_b", [cfg.ROWSB, DH], F8)
    h2ta = nc.dram_tensor("h2ta", [cfg.NA, DH], F8, addr_space="Shared")
    h2tb = nc.dram_tensor("h2tb", [cfg.NB, DH], F8, addr_space="Shared")
    pool_part = nc.dram_tensor("pool_part", [G, 2 * P], F32)
    pool_red = nc.dram_tensor("pool_red", [GSH, 2 * P], F32)

    la_tile = [None]

    def selu3(scal, vec, out_ap, z_ap, tmp_pool, shape):
        """out = selu(z) = lam*Relu(z) - lam*a*Relu(1 - exp(z))."""
        r1 = tmp_pool.tile(shape, F32, tag="selu_r1")
        e = tmp_pool.tile(shape, F32, tag="selu_e")
        r2 = tmp_pool.tile(shape, F32, tag="selu_r2")
        scal.activation(r1[:], z_ap, AF.Relu, scale=SELU_LAM)
        scal.activation(e[:], z_ap, AF.Exp)
        scal.activation(r2[:], e[:], AF.Relu, bias=la_tile[0][:shape[0], 0:1],
                        scale=-SELU_LA)
        vec.tensor_tensor(out_ap, r1[:], r2[:], OP.subtract)

    def agg_matmuls(psum_ap, stat3, mov3, runs, base):
        """Accumulate one-hot aggregation over this window's slot runs using
        DoubleRow pairs with plain-fp8 leftovers."""
        calls = []
        for (off, n) in runs:
            sl = off  # group-local
            for t in range(0, n - 1, 2):
                calls.append((sl + t, 2))
            if n % 2:
                calls.append((sl + n - 1, 1))
        nb = len(calls)
        for i, (sl, cnt) in enumerate(calls):
            if cnt == 2:
                nc.tensor.matmul(
                    psum_ap, stat3[:, sl: sl + 2, :], mov3[:, sl: sl + 2, :],
                    start=(i == 0), stop=(i == nb - 1), perf_mode=DR,
                )
            else:
                nc.tensor.matmul(
                    psum_ap, stat3[:, sl, :], mov3[:, sl, :],
                    start=(i == 0), stop=(i == nb - 1),
                )

    with tile.TileContext(nc) as tc:
        with (
            tc.tile_pool(name="consts", bufs=1) as cpool,
            tc.tile_pool(name="idxpool", bufs=1) as ipool,
            tc.tile_pool(name="atab", bufs=2) as apool,
            tc.tile_pool(name="gx2", bufs=3) as gx2pool,
            tc.tile_pool(name="oh", bufs=3) as ohpool,
            tc.tile_pool(name="work", bufs=3) as wpool,
            tc.tile_pool(name="head", bufs=1) as hpool,
            tc.tile_pool(name="post", bufs=2) as ppool,
            tc.tile_pool(name="ps_sm", bufs=2, space="PSUM") as ps_sm,
            tc.tile_pool(name="ps_h1", bufs=2, space="PSUM") as ps_h1,
            tc.tile_pool(name="ps_h2", bufs=2, space="PSUM") as ps_h2,
            tc.tile_pool(name="ps_pool", bufs=1, space="PSUM") as ps_pool,
        ):
            def load(pool, dram, shape, dt):
                t = pool.tile(shape, dt, tag=dram.name + "_sb")
                nc.sync.dma_start(out=t[:], in_=dram[tuple(slice(0, s) for s in shape)])
                return t

            la = cpool.tile([P, 1], F32, tag="la_const")
            nc.vector.memset(la[:], SELU_LA)
            la_tile[0] = la

            idx_sb = load(ipool, idxs, [P, cfg.IDX_COLS], I16)
            dinv_sb = load(cpool, dinv_d, [P, W], F32)
            W1p_sb = load(cpool, W1p_d, [16, DH], BF16)
            W2_sb = load(cpool, W2_d, [P, 2 * DH], BF16)
            b1_sb = load(cpool, b1_d, [P, 2], F32) if has_b1 else None
            b2b_sb = load(cpool, b2b_d, [P, DH], F32) if has_b2 else None
            fc1_sb = load(cpool, fc1_d, [P, 2 * cfg.DFC], F32)
            fc1b_sb = load(cpool, fc1b_d, [GSH, cfg.DFC], F32)
            fc2_sb = load(cpool, fc2_d, [cfg.DFC, cfg.NCLS], F32)
            fc2b_sb = load(cpool, fc2b_d, [GSH, cfg.NCLS], F32)
            ident_sb = load(cpool, ident_d, [P, P], F32)
            cntinv_sb = load(cpool, cntinv_d, [GSH, 1], F32)

            def load_onehots(g):
                base = cfg.grp_slot_base[g]
                ns = cfg.grp_nslots[g]
                ohsl = ohpool.tile([P, ns, P], F8, tag="ohslab")
                nc.sync.dma_start(
                    out=ohsl[:], in_=oht_d[:, base * P: (base + ns) * P])
                return ohsl

            # ================= Phase A: layer 1 -> shard_a / shard_b ========
            for g, (w0, wg) in enumerate(cfg.groups):
                base = cfg.grp_slot_base[g]
                ns = cfg.grp_nslots[g]
                at = apool.tile([P, ns, 16], F8, tag="atab_t")
                nc.sync.dma_start(out=at[:], in_=atab_d[:, base * 16: (base + ns) * 16])
                ohsl = load_onehots(g)
                for k in range(wg):
                    w = w0 + k
                    runs = _win_runs(cfg, g, k)
                    psA = ps_sm.tile([16, P], F32, tag="sm")
                    agg_matmuls(psA[:], at, ohsl, runs, base)
                    aggT = wpool.tile([16, P], BF16, tag="aggT")
                    nc.scalar.copy(aggT[:], psA[:])
                    ph1 = ps_h1.tile([P, DH], F32, tag="ph1")
                    for j in range(2):
                        nc.tensor.matmul(
                            ph1[:, j * P: (j + 1) * P],
                            W1p_sb[:, j * P: (j + 1) * P], aggT[:],
                            start=True, stop=True,
                        )
                    h1T = ppool.tile([P, DH], BF16, tag="a_h1T")
                    if has_b1:
                        r1 = ppool.tile([P, DH], F32, tag="a_r1")
                        e = ppool.tile([P, DH], F32, tag="a_e")
                        r2 = ppool.tile([P, DH], F32, tag="a_r2")
                        for j in range(2):
                            sl_ = slice(j * P, (j + 1) * P)
                            nc.scalar.activation(r1[:, sl_], ph1[:, sl_], AF.Relu,
                                                 bias=b1_sb[:, j: j + 1],
                                                 scale=SELU_LAM)
                            nc.scalar.activation(e[:, sl_], ph1[:, sl_], AF.Exp,
                                                 bias=b1_sb[:, j: j + 1])
                        nc.scalar.activation(r2[:], e[:], AF.Relu,
                                             bias=la_tile[0][:, 0:1],
                                             scale=-SELU_LA)
                        nc.vector.tensor_tensor(h1T[:], r1[:], r2[:], OP.subtract)
                    else:
                        selu3(nc.scalar, nc.vector, h1T[:], ph1[:], ppool, [P, DH])

                    psum_h2t = ps_h2.tile([P, DH], F32, tag="main")
                    for j in range(2):
                        nc.tensor.matmul(
                            psum_h2t[:], h1T[:, j * P: (j + 1) * P],
                            W2_sb[:, j * DH: (j + 1) * DH],
                            start=(j == 0), stop=(j == 1),
                        )
                    h2tw = ppool.tile([P, DH], F8, tag="h2tw")
                    nc.scalar.activation(h2tw[:], psum_h2t[:], AF.Copy,
                                         scale=dinv_sb[:, w: w + 1])
                    rows = min(P, cfg.NSH - w * P)
                    if w < cfg.WA:
                        nc.sync.dma_start(out=shard_a[w * P: w * P + rows, :],
                                          in_=h2tw[:rows, :])
                    else:
                        r0 = w * P - cfg.ROWSA
                        nc.sync.dma_start(out=shard_b[r0: r0 + rows, :],
                                          in_=h2tw[:rows, :])
                if w0 + wg == cfg.WA:
                    nc.gpsimd.collective_compute(
                        "AllGather", OP.bypass,
                        replica_groups=[list(range(NCORES))],
                        ins=[shard_a[:, :]], outs=[h2ta[:, :]],
                    )
            nc.gpsimd.collective_compute(
                "AllGather", OP.bypass,
                replica_groups=[list(range(NCORES))],
                ins=[shard_b[:, :]], outs=[h2tb[:, :]],
            )

            # ================= Phase B: layer 2 + pooling ===================
            ppg0 = ps_pool.tile([P, DH], F32, tag="ppg0")
            ppg1 = ps_pool.tile([P, DH], F32, tag="ppg1")
            ppgs = [ppg0, ppg1]
            qtab = [h2ta, h2ta, h2tb, h2tb]
            for g, (w0, wg) in enumerate(cfg.groups):
                base = cfg.grp_slot_base[g]
                ns = cfg.grp_nslots[g]
                gt2 = gx2pool.tile([P, ns, DH], F8, tag="gx2_t")
                for q in range(NQ):
                    nq = cfg.grp_q_n[g][q]
                    if nq == 0:
                        continue
                    s0 = cfg.grp_q_off[g][q]
                    tab = qtab[q]
                    nc.gpsimd.dma_gather(
                        gt2[:, s0: s0 + nq, :],
                        tab[:, :],
                        idx_sb[:, cfg.grp_idx_col[g][q]:
                               cfg.grp_idx_col[g][q] + nq * 8],
                        nq * P, nq * P, DH,
                        single_packet=False, queue_num=q,
                    )
                ohsl = load_onehots(g)
                ohg_sl = ohpool.tile([P, wg * G], BF16, tag="ohg_slab")
                nc.sync.dma_start(out=ohg_sl[:], in_=ohgt_d[:, w0 * G: (w0 + wg) * G])
                for k in range(wg):
                    w = w0 + k
                    runs = _win_runs(cfg, g, k)
                    psum2 = ps_h2.tile([P, DH], F32, tag="main")
                    agg_matmuls(psum2[:], ohsl, gt2, runs, base)
                    zd = ppool.tile([P, DH], F32, tag="b_zd")
                    nc.scalar.activation(zd[:], psum2[:], AF.Copy,
                                         scale=dinv_sb[:, w: w + 1])
                    if has_b2:
                        zb2 = ppool.tile([P, DH], F32, tag="b_zb2")
                        nc.vector.tensor_tensor(zb2[:], zd[:], b2b_sb[:], OP.add)
                        zd = zb2
                    h2w = ppool.tile([P, DH], BF16, tag="b_h2w")
                    selu3(nc.scalar, nc.vector, h2w[:], zd[:], ppool, [P, DH])
                    for j in range(2):
                        nc.tensor.matmul(
                            ppgs[j][:], ohg_sl[:, k * G + j * P: k * G + (j + 1) * P],
                            h2w[:],
                            start=(w == 0), stop=(w == W - 1),
                        )

            # ================= pooled head (per-core slice of graphs) =======
            pT0 = hpool.tile([P, DH], F32, tag="pT0")
            pT1 = hpool.tile([P, DH], F32, tag="pT1")
            nc.scalar.copy(pT0[:], ppg0[:])
            nc.scalar.copy(pT1[:], ppg1[:])
            nc.sync.dma_start(out=pool_part[0:P, :], in_=pT0[:])
            nc.sync.dma_start(out=pool_part[P: 2 * P, :], in_=pT1[:])
            nc.gpsimd.collective_compute(
                "ReduceScatter", OP.add,
                replica_groups=[list(range(NCORES))],
                ins=[pool_part[:, :]], outs=[pool_red[:, :]],
            )
            psr = hpool.tile([GSH, 2 * P], F32, tag="psr")
            nc.sync.dma_start(out=psr[:], in_=pool_red[:, :])
            pm = hpool.tile([GSH, 2 * P], F32, tag="pm")
            nc.scalar.activation(pm[:], psr[:], AF.Copy,
                                 scale=cntinv_sb[:, 0:1])
            gsel = hpool.tile([GSH, 2 * P], F32, tag="gsel")
            selu3(nc.scalar, nc.vector, gsel[:], pm[:], hpool, [GSH, 2 * P])

            gT = hpool.tile([P, 2 * GSH], F32, tag="gT")
            for j in range(2):
                psT = ps_sm.tile([P, GSH], F32, tag="sm")
                nc.tensor.transpose(psT[:, :], gsel[:, j * P: (j + 1) * P],
                                    ident_sb[0:GSH, 0:GSH])
                nc.scalar.copy(gT[:, j * GSH: (j + 1) * GSH], psT[:])
            psum_fc1 = ps_h2.tile([GSH, cfg.DFC], F32, tag="main")
            for j in range(2):
                nc.tensor.matmul(
                    psum_fc1[:], gT[:, j * GSH: (j + 1) * GSH],
                    fc1_sb[:, j * cfg.DFC: (j + 1) * cfg.DFC],
                    start=(j == 0), stop=(j == 1),
                )
            zf = hpool.tile([GSH, cfg.DFC], F32, tag="zf")
            nc.vector.tensor_tensor(zf[:], psum_fc1[:], fc1b_sb[:], OP.add)
            hsel = hpool.tile([GSH, cfg.DFC], F32, tag="hsel")
            selu3(nc.scalar, nc.vector, hsel[:], zf[:], hpool, [GSH, cfg.DFC])

            psT2 = ps_sm.tile([cfg.DFC, GSH], F32, tag="sm")
            nc.tensor.transpose(psT2[:], hsel[:], ident_sb[0:GSH, 0:GSH])
            hT = hpool.tile([cfg.DFC, GSH], F32, tag="hT")
            nc.scalar.copy(hT[:], psT2[:])
            psum_fc2 = ps_sm.tile([GSH, cfg.NCLS], F32, tag="sm")
            nc.tensor.matmul(psum_fc2[:], hT[:], fc2_sb[:], start=True, stop=True)
            lg = hpool.tile([GSH, cfg.NCLS], F32, tag="lg")
            nc.vector.tensor_tensor(lg[:], psum_fc2[:], fc2b_sb[:], OP.add)

            nm = hpool.tile([GSH, 1], F32, tag="nm")
            nc.vector.tensor_reduce(nm[:], lg[:], mybir.AxisListType.X, OP.max,
                                    negate=True)
            e4 = hpool.tile([GSH, cfg.NCLS], F32, tag="e4")
            nc.scalar.activation(e4[:], lg[:], AF.Exp, bias=nm[:, 0:1])
            s4 = hpool.tile([GSH, 1], F32, tag="s4")
            nc.vector.tensor_reduce(s4[:], e4[:], mybir.AxisListType.X, OP.add)
            ls = hpool.tile([GSH, 1], F32, tag="ls")
            nc.scalar.activation(ls[:], s4[:], AF.Ln)
            q_ = hpool.tile([GSH, 1], F32, tag="q")
            nc.vector.tensor_tensor(q_[:], nm[:], ls[:], OP.subtract)
            outj = hpool.tile([GSH, cfg.NCLS], F32, tag="outj")
            nc.vector.tensor_scalar(outj[:], lg[:], q_[:, 0:1], None, OP.add)
            nc.sync.dma_start(out=out_d[0:GSH, :], in_=outj[:, :])

    nc.compile()
    return nc


_CACHE = {}


def run_gcn(inputs, n_nodes, n_graphs, d_in=14, d_hid=256, d_fc=128, n_cls=2,
            grp=3, trace=False):
    cl = CfgLike(n_nodes, grp)
    s, d = sort_edges(inputs, n_nodes)
    th_cw, cut = compute_tile_budget(cl, s, d, n_nodes, n_nodes // NCORES)
    cfg = Cfg(n_nodes, n_graphs, d_in, d_hid, d_fc, n_cls, th_cw, grp)
    per_core, shared = host_prep(inputs, cfg, s, d, cut)
    key = (n_nodes, n_graphs, grp, shared["has_b1"], shared["has_b2"],
           tuple(tuple(t) for t in cfg.TH))
    if key not in _CACHE:
        _CACHE[key] = build_nc(cfg, shared["has_b1"], shared["has_b2"])
    nc = _CACHE[key]
    res = run_bass_kernel_spmd(nc, per_core, list(range(NCORES)), trace=trace)
    out = np.concatenate(
        [np.asarray(res.results[r]["out"][0: cfg.GSH])
         for r in range(NCORES)], axis=0).astype(np.float32)
    return out, res


def kernel(**inputs) -> np.ndarray:
    out, _ = run_gcn(
        inputs, n_nodes=50000, n_graphs=256,
        trace=bool(int(os.environ.get("GCN_TRACE", "0"))),
    )
    return out
